# revision 25
# baseline (speedup 1.0000x reference)
"""Trainium2 Bass kernel: BasicLSTMActorCritic, data-parallel over batch on 8 cores.

Per-core shard: B=32 of 256.  T=512, B=256, O=720, H=128, A=2.
Design:
  - Host prep: gate columns permuted to [i, f, o, g]; g-gate weights scaled x2 so
    tanh(z) = 2*sigmoid(2z)-1 means ONE sigmoid covers all 4 gates.  Bias b
    folded into Wx as an extra contraction row (ones row in x^T on chip).
  - All-bf16 x pipeline: x DMA-cast f32->bf16 on load (SWDGE), PE-transposed
    to x^T, bf16 xW matmuls accumulate z^T directly in PSUM (f32).
  - Recurrence: 512 serial steps, batch split into 2 phase-shifted half-chains
    of 16 so the two chains' stages overlap across engines.  Per chain-step:
    4 bf16 matmuls (Wh stationary, h moving) accumulate onto the PSUM z,
    one sigmoid (ACT) for all 4 gates, then on DVE: u = CMUL(sg, si) =
    (2*sg-1)*si = tanh(zg)*si, e = sf*c, c = e+u, and h = CMUL(sigmoid(2c), so)
    = tanh(c)*so — the custom CMUL op removes the ACT tanh entirely.
  - PSUM->SBUF x^T copies ride DMA (SP HWDGE), not DVE.
  - Heads (actor/critic MLPs) computed once at the end from final h.
"""

import sys

sys.path.insert(0, "/opt/trn_rl_repo")

import numpy as np

T, B, O, H, A = 512, 256, 720, 128, 2
NCORES = 8
BS = B // NCORES  # 32 batch per core
GN = 2  # phase-shifted half-chains
BG = BS // GN  # 16 batch per chain
G4 = 4 * H  # 512
TS = 8  # timesteps per block
NB = T // TS  # 64 blocks
RB = TS * BS  # 256 moving cols per block
KT = 6  # K tiles over the padded contraction dim
# last k-tile rows: [80 x rows; 16 zero rows; bias row at 96; 31 zero rows] —
# engine APs need base partition 0 (any count) or 32/64/96 (count <= 32)
KSZ = [128, 128, 128, 128, 128, 128]

_nc_cache = {}


def _register_cmul():
    from concourse import dve_ops
    from concourse.dve_spec import Spec, Src0, Src1, One, lower
    from concourse.dve_spec import _has_src1 as has_src1
    from concourse.dve_uop import DveOpSpec

    for o in dve_ops.OPS:
        if o.name == "ANT_LSTM_CMUL":
            return o
    spec = Spec(
        body=(Src0 + Src0 - One) * Src1,
        reference=lambda in0, in1: (2.0 * in0 - 1.0) * in1,
    )
    opcode = dve_ops._CUSTOM_DVE_ROW_BASE + len(dve_ops.OPS)
    shas = {}
    for ver in ("v3", "v4"):
        uops = lower(spec, ver=ver)
        shas[ver] = DveOpSpec(
            name="ANT_LSTM_CMUL", opcode=opcode, uops=uops, rd1_en=has_src1(spec)
        ).sha(ver)
    op = dve_ops.DveOp("ANT_LSTM_CMUL", spec, subdim=False, uops_sha=shas)
    dve_ops.OPS.append(op)
    dve_ops._SUB_OPCODE_FOR_NAME[op.name] = opcode
    return op


def _build(nb=NB, dbg=False):
    import concourse.tile as tile
    from concourse.tile_rust import add_dep_helper
    from concourse import bacc, mybir
    from contextlib import ExitStack

    f32 = mybir.dt.float32
    bf16 = mybir.dt.bfloat16
    AF = mybir.ActivationFunctionType
    CMUL = _register_cmul()

    nc = bacc.Bacc("TRN2", target_bir_lowering=False, debug=False)

    # ---- I/O ----
    x_d = nc.dram_tensor("x", [nb * TS, BS, O], f32, kind="ExternalInput")
    h0_d = nc.dram_tensor("h0T", [H, BS], f32, kind="ExternalInput")
    c0_d = nc.dram_tensor("c0T", [H, BS], f32, kind="ExternalInput")
    wx_d = nc.dram_tensor("Wxp", [sum(KSZ), G4], f32, kind="ExternalInput")
    wh_d = nc.dram_tensor("Whp", [H, G4], f32, kind="ExternalInput")
    eye_d = nc.dram_tensor("eye", [128, 128], f32, kind="ExternalInput")
    kp_d = nc.dram_tensor("kpad", [128, RB], f32, kind="ExternalInput")
    wa1_d = nc.dram_tensor("Wa1", [H, H], f32, kind="ExternalInput")
    ba1_d = nc.dram_tensor("ba1", [H, 1], f32, kind="ExternalInput")
    wa2_d = nc.dram_tensor("Wa2", [H, A], f32, kind="ExternalInput")
    ba2_d = nc.dram_tensor("ba2", [A, 1], f32, kind="ExternalInput")
    wc1_d = nc.dram_tensor("Wc1", [H, H], f32, kind="ExternalInput")
    bc1_d = nc.dram_tensor("bc1", [H, 1], f32, kind="ExternalInput")
    wc2_d = nc.dram_tensor("Wc2", [H, 1], f32, kind="ExternalInput")
    bc2_d = nc.dram_tensor("bc2", [1, 1], f32, kind="ExternalInput")
    ls_d = nc.dram_tensor("logstd", [A, 1], f32, kind="ExternalInput")

    oh_d = nc.dram_tensor("out_h", [H, BS], f32, kind="ExternalOutput")
    oc_d = nc.dram_tensor("out_c", [H, BS], f32, kind="ExternalOutput")
    om_d = nc.dram_tensor("out_m", [A, BS], f32, kind="ExternalOutput")
    ov_d = nc.dram_tensor("out_v", [1, BS], f32, kind="ExternalOutput")
    os_d = nc.dram_tensor("out_s", [A, 1], f32, kind="ExternalOutput")

    if dbg:
        od1_d = nc.dram_tensor("out_sig0", [H, 4 * BG], f32, kind="ExternalOutput")
        od2_d = nc.dram_tensor("out_c1", [H, BS], f32, kind="ExternalOutput")

    with tile.TileContext(nc) as tc, ExitStack() as ctx:
        const = ctx.enter_context(tc.tile_pool(name="const", bufs=1))
        natp = ctx.enter_context(tc.tile_pool(name="nat", bufs=3))
        xtp = ctx.enter_context(tc.tile_pool(name="xt", bufs=1))
        sigp = ctx.enter_context(tc.tile_pool(name="sig", bufs=4))
        vecp = ctx.enter_context(tc.tile_pool(name="vec", bufs=3))
        hbp = ctx.enter_context(tc.tile_pool(name="hb", bufs=3))
        pzp = ctx.enter_context(tc.tile_pool(name="pz", bufs=2, space="PSUM"))
        ptrp = ctx.enter_context(tc.tile_pool(name="ptr", bufs=2, space="PSUM"))
        phdp = ctx.enter_context(tc.tile_pool(name="phd", bufs=1, space="PSUM"))

        dma = nc.sync.dma_start
        cdma = nc.gpsimd.dma_start  # SWDGE: casts f32->bf16 during transfer

        # ---- constants / weights to SBUF ----
        ls_sb = const.tile([A, 1], f32, tag="ls")
        dma(out=ls_sb[:], in_=ls_d.ap())
        std_sb = const.tile([A, 1], f32, tag="std")
        # exp FIRST on ACT (exp table), then everything else uses sigmoid table
        nc.scalar.activation(std_sb[:], ls_sb[:], AF.Exp)
        dma(out=os_d.ap(), in_=std_sb[:])

        eye = const.tile([128, 128], bf16, tag="eye")
        cdma(out=eye[:], in_=eye_d.ap())

        wx_sb = []
        r0 = 0
        for kt in range(KT):
            t_ = const.tile([KSZ[kt], G4], bf16, tag=f"wx{kt}")
            cdma(out=t_[:], in_=wx_d.ap()[r0 : r0 + KSZ[kt], :])
            wx_sb.append(t_)
            r0 += KSZ[kt]

        wh_b = const.tile([H, G4], bf16, tag="whb")
        cdma(out=wh_b[:], in_=wh_d.ap())

        def load_cast(d, shape, tag):
            tb = const.tile(shape, bf16, tag=tag + "b")
            cdma(out=tb[:], in_=d.ap())
            return tb

        wa1_b = load_cast(wa1_d, [H, H], "wa1")
        wa2_b = load_cast(wa2_d, [H, A], "wa2")
        wc1_b = load_cast(wc1_d, [H, H], "wc1")
        wc2_b = load_cast(wc2_d, [H, 1], "wc2")
        ba1_sb = const.tile([H, 1], f32, tag="ba1")
        dma(out=ba1_sb[:], in_=ba1_d.ap())
        ba2_sb = const.tile([A, 1], f32, tag="ba2")
        dma(out=ba2_sb[:], in_=ba2_d.ap())
        bc1_sb = const.tile([H, 1], f32, tag="bc1")
        dma(out=bc1_sb[:], in_=bc1_d.ap())
        bc2_sb = const.tile([1, 1], f32, tag="bc2")
        dma(out=bc2_sb[:], in_=bc2_d.ap())

        # ---- state: c [128, 32] f32 (chain q owns cols 16q:16q+16);
        #      h per-chain bf16 tiles ----
        h0f = const.tile([H, BS], f32, tag="h0f")
        dma(out=h0f[:], in_=h0_d.ap())
        hq = []
        for q in range(GN):
            h0b = const.tile([H, BG], bf16, tag=f"h0b{q}")
            nc.vector.tensor_copy(h0b[:], h0f[:, q * BG : (q + 1) * BG])
            hq.append(h0b)
        cT = const.tile([H, BS], f32, tag="cT")
        dma(out=cT[:], in_=c0_d.ap())

        # xT double buffers; k-tile 5 pad: zeros at 80:96, ones at 96:128 so
        # the bias row at weight-partition 96 contributes b, rest 0
        xt_bufs = []
        for j in range(2):
            xb = xtp.tile([128, KT * RB], bf16, tag=f"xtb{j}")
            cdma(out=xb[:, 5 * RB : 6 * RB], in_=kp_d.ap())
            xt_bufs.append(xb)

        sig_last = [None, None]
        sc_last = [None, None]
        last_sig_a = None

        pz_of = {}

        def prep_gen(blk):
            """Emit block `blk`'s input pipeline (DMA, transposes, PSUM->SBUF
            copies, xW matmuls) in small units; the caller interleaves these
            between recurrence-step emissions so the PE stream has no long
            idle stretches (keeps the HAM clock warm)."""
            t0 = blk * TS
            xt = xt_bufs[blk % 2]
            nats = []
            for r in range(2):
                nat = natp.tile([128, O], bf16, tag="nat")
                src = x_d.ap()[t0 + 4 * r : t0 + 4 * r + 4, :, :]
                cdma(out=nat[:], in_=src.rearrange("a b c -> (a b) c"))
                nats.append(nat)
            yield
            for kt in range(KT):
                cw = 80 if kt == 5 else 128
                ptr = ptrp.tile([128, 256], bf16, tag="ptr")
                for r in range(2):
                    nc.tensor.transpose(
                        ptr[0:cw, 128 * r : 128 * r + 128],
                        nats[r][:, 128 * kt : 128 * kt + cw],
                        eye[:],
                    )
                if kt % 2 == 0:
                    nc.vector.tensor_copy(
                        xt[0:cw, kt * RB : kt * RB + RB], ptr[0:cw, :]
                    )
                else:
                    nc.scalar.copy(
                        xt[0:cw, kt * RB : kt * RB + RB], ptr[0:cw, :]
                    )
                yield
            pz = pzp.tile([128, 4 * RB], f32, tag="pz")
            pz_of[blk] = pz
            for g in range(4):
                for kt in range(KT):
                    nc.tensor.matmul(
                        pz[:, g * RB : (g + 1) * RB],
                        wx_sb[kt][:, g * H : (g + 1) * H],
                        xt[0 : KSZ[kt], kt * RB : (kt + 1) * RB],
                        # start=True clears has_written for the WHOLE bank:
                        # issue exactly once per bank (gates 0/2 lead banks 0/1)
                        start=(kt == 0 and g in (0, 2)),
                        stop=False,
                        skip_group_check=True,
                    )
                    if kt % 2 == 1:
                        yield

        for _ in prep_gen(0):
            pass

        for blk in range(nb):
            pz = pz_of[blk]
            gen = prep_gen(blk + 1) if blk + 1 < nb else None

            # recurrence: 2 phase-shifted half-chains of 16; chain B emitted
            # one step behind chain A so their stages overlap across engines
            steps = []
            for t in range(TS):
                steps.append((t, 0))
                if blk == 0 and t == 0:
                    pass
                elif t == 0:
                    steps.append((TS - 1, 1, blk - 1))
                else:
                    steps.append((t - 1, 1))
            if blk == nb - 1:
                steps.append((TS - 1, 1))
            for st in steps:
                if len(st) == 3:
                    t, q, sblk = st
                    spz = pz_of[blk - 1]
                else:
                    t, q = st
                    spz = pz
                if True:
                    co = 32 * t + BG * q
                    mms = []
                    for g in range(4):
                        mm = nc.tensor.matmul(
                            spz[:, g * RB + co : g * RB + co + BG],
                            wh_b[:, g * H : (g + 1) * H],
                            hq[q][:],
                            start=False,
                            stop=(t == TS - 1),
                            skip_group_check=True,
                        )
                        mms.append(mm)
                    if q == 1 and last_sig_a is not None:
                        add_dep_helper(
                            mms[0].ins, last_sig_a.ins,
                            reason="anti-phase chains",
                        )
                    sig = sigp.tile([128, 4 * BG], bf16, tag=f"sig{q}")
                    zview = spz[:].rearrange("p (g c) -> p g c", g=4)[
                        :, :, co : co + BG
                    ]
                    sview = sig[:].rearrange("p (g c) -> p g c", g=4)
                    sig_i = nc.scalar.activation(sview, zview, AF.Sigmoid)
                    if q == 0:
                        last_sig_a = sig_i
                    s_i = sig[:, 0:BG]
                    s_f = sig[:, BG : 2 * BG]
                    s_o = sig[:, 2 * BG : 3 * BG]
                    s_g = sig[:, 3 * BG : 4 * BG]
                    cq = cT[:, q * BG : (q + 1) * BG]
                    u = vecp.tile([H, BG], f32, tag=f"u{q}")
                    nc.vector._custom_dve(CMUL, out=u[:], in0=s_g, in1=s_i)
                    e = vecp.tile([H, BG], f32, tag=f"e{q}")
                    nc.vector.tensor_mul(e[:], s_f, cq)
                    nc.vector.tensor_add(cq, e[:], u[:])
                    sc = vecp.tile([H, BG], bf16, tag=f"sc{q}")
                    nc.scalar.activation(sc[:], cq, AF.Sigmoid, scale=2.0)
                    hn = hbp.tile([H, BG], bf16, tag=f"h{q}")
                    nc.vector._custom_dve(CMUL, out=hn[:], in0=sc[:], in1=s_o)
                    hq[q] = hn
                    if blk == nb - 1 and t == TS - 1:
                        sig_last[q] = sig
                        sc_last[q] = sc
                if gen is not None:
                    for _ in range(2):
                        if next(gen, "done") == "done":
                            gen = None
                            break
            while gen is not None and next(gen, "done") != "done":
                pass

        # ---- outputs ----
        hf = const.tile([H, BS], f32, tag="hf")
        hb_all = const.tile([H, BS], bf16, tag="hball")
        for q in range(GN):
            nc.vector._custom_dve(
                CMUL,
                out=hf[:, q * BG : (q + 1) * BG],
                in0=sc_last[q][:],
                in1=sig_last[q][:, 2 * BG : 3 * BG],
            )
            nc.vector.tensor_copy(hb_all[:, q * BG : (q + 1) * BG], hq[q][:])
        dma(out=oh_d.ap(), in_=hf[:])
        dma(out=oc_d.ap(), in_=cT[:])

        # actor head
        p1 = phdp.tile([H, BS], f32, tag="ph")
        nc.tensor.matmul(p1[:], wa1_b[:], hb_all[:], start=True, stop=True)
        a1 = const.tile([H, BS], bf16, tag="a1")
        nc.scalar.activation(a1[:], p1[:], AF.Tanh, bias=ba1_sb[:, 0:1])
        p2 = phdp.tile([A, BS], f32, tag="ph2")
        nc.tensor.matmul(p2[:], wa2_b[:], a1[:], start=True, stop=True)
        m_sb = const.tile([A, BS], f32, tag="msb")
        nc.scalar.activation(m_sb[:], p2[:], AF.Identity, bias=ba2_sb[:, 0:1])
        dma(out=om_d.ap(), in_=m_sb[:])

        # critic head
        q1 = phdp.tile([H, BS], f32, tag="ph")
        nc.tensor.matmul(q1[:], wc1_b[:], hb_all[:], start=True, stop=True)
        c1 = const.tile([H, BS], bf16, tag="c1")
        nc.scalar.activation(c1[:], q1[:], AF.Tanh, bias=bc1_sb[:, 0:1])
        q2 = phdp.tile([1, BS], f32, tag="ph2")
        nc.tensor.matmul(q2[:], wc2_b[:], c1[:], start=True, stop=True)
        v_sb = const.tile([1, BS], f32, tag="vsb")
        nc.scalar.activation(v_sb[:], q2[:], AF.Identity, bias=bc2_sb[:, 0:1])
        dma(out=ov_d.ap(), in_=v_sb[:])

    nc.compile()
    return nc


def kernel(x, h0, c0, Wx, Wh, b, Wa1, ba1, Wa2, ba2, log_std, Wc1, bc1, Wc2, bc2):
    from concourse.bass_utils import run_bass_kernel_spmd

    x = np.asarray(x, np.float32)
    h0 = np.asarray(h0, np.float32)
    c0 = np.asarray(c0, np.float32)
    Wx = np.asarray(Wx, np.float32)
    Wh = np.asarray(Wh, np.float32)
    b = np.asarray(b, np.float32)

    # gate order in reference: [i, f, g, o]; ours: [i, f, o, g] with g scaled x2
    perm = np.concatenate(
        [np.arange(0, H), np.arange(H, 2 * H), np.arange(3 * H, 4 * H),
         np.arange(2 * H, 3 * H)]
    )
    Wxp = np.vstack(
        [Wx, np.zeros((16, 4 * H), np.float32), b[None, :],
         np.zeros((31, 4 * H), np.float32)]
    )[:, perm].copy()
    Wxp[:, 3 * H :] *= 2.0
    Whp = Wh[:, perm].copy()
    Whp[:, 3 * H :] *= 2.0

    if "nc" not in _nc_cache:
        _nc_cache["nc"] = _build()
    nc = _nc_cache["nc"]

    common = {
        "Wxp": np.ascontiguousarray(Wxp),
        "Whp": np.ascontiguousarray(Whp),
        "eye": np.eye(128, dtype=np.float32),
        "kpad": np.vstack([np.zeros((96, RB), np.float32),
                           np.ones((32, RB), np.float32)]),
        "Wa1": np.asarray(Wa1, np.float32),
        "ba1": np.asarray(ba1, np.float32).reshape(H, 1),
        "Wa2": np.asarray(Wa2, np.float32),
        "ba2": np.asarray(ba2, np.float32).reshape(A, 1),
        "Wc1": np.asarray(Wc1, np.float32),
        "bc1": np.asarray(bc1, np.float32).reshape(H, 1),
        "Wc2": np.asarray(Wc2, np.float32),
        "bc2": np.asarray(bc2, np.float32).reshape(1, 1),
        "logstd": np.asarray(log_std, np.float32).reshape(A, 1),
    }
    in_maps = []
    for c in range(NCORES):
        bs = slice(c * BS, (c + 1) * BS)
        m = dict(common)
        m["x"] = np.ascontiguousarray(x[:, bs, :])
        m["h0T"] = np.ascontiguousarray(h0[bs].T)
        m["c0T"] = np.ascontiguousarray(c0[bs].T)
        in_maps.append(m)

    res = run_bass_kernel_spmd(nc, in_maps, core_ids=list(range(NCORES)))
    rs = res.results

    hT_full = np.concatenate([rs[c]["out_h"].T for c in range(NCORES)], 0)
    cT_full = np.concatenate([rs[c]["out_c"].T for c in range(NCORES)], 0)
    mean = np.concatenate([rs[c]["out_m"].T for c in range(NCORES)], 0)
    value = np.concatenate([rs[c]["out_v"][0] for c in range(NCORES)], 0)
    std = rs[0]["out_s"][:, 0]
    return mean, std, value, hT_full, cT_full


# revision 26
# speedup vs baseline: 1.0005x; 1.0005x over previous
"""Trainium2 Bass kernel: BasicLSTMActorCritic, data-parallel over batch on 8 cores.

Per-core shard: B=32 of 256.  T=512, B=256, O=720, H=128, A=2.
Design:
  - Host prep: gate columns permuted to [i, f, o, g]; g-gate weights scaled x2 so
    tanh(z) = 2*sigmoid(2z)-1 means ONE sigmoid covers all 4 gates.  Bias b
    folded into Wx as an extra contraction row (ones row in x^T on chip).
  - All-bf16 x pipeline: x DMA-cast f32->bf16 on load (SWDGE), PE-transposed
    to x^T, bf16 xW matmuls accumulate z^T directly in PSUM (f32).
  - Recurrence: 512 serial steps, batch split into 2 phase-shifted half-chains
    of 16 so the two chains' stages overlap across engines.  Per chain-step:
    4 bf16 matmuls (Wh stationary, h moving) accumulate onto the PSUM z,
    one sigmoid (ACT) for all 4 gates, then on DVE: u = CMUL(sg, si) =
    (2*sg-1)*si = tanh(zg)*si, e = sf*c, c = e+u, and h = CMUL(sigmoid(2c), so)
    = tanh(c)*so — the custom CMUL op removes the ACT tanh entirely.
  - PSUM->SBUF x^T copies ride DMA (SP HWDGE), not DVE.
  - Heads (actor/critic MLPs) computed once at the end from final h.
"""

import sys

sys.path.insert(0, "/opt/trn_rl_repo")

import numpy as np

T, B, O, H, A = 512, 256, 720, 128, 2
NCORES = 8
BS = B // NCORES  # 32 batch per core
GN = 2  # phase-shifted half-chains
BG = BS // GN  # 16 batch per chain
G4 = 4 * H  # 512
TS = 8  # timesteps per block
NB = T // TS  # 64 blocks
RB = TS * BS  # 256 moving cols per block
KT = 6  # K tiles over the padded contraction dim
# last k-tile rows: [80 x rows; 16 zero rows; bias row at 96; 31 zero rows] —
# engine APs need base partition 0 (any count) or 32/64/96 (count <= 32)
KSZ = [128, 128, 128, 128, 128, 128]

_nc_cache = {}


def _register_cmul():
    from concourse import dve_ops
    from concourse.dve_spec import Spec, Src0, Src1, One, lower
    from concourse.dve_spec import _has_src1 as has_src1
    from concourse.dve_uop import DveOpSpec

    for o in dve_ops.OPS:
        if o.name == "ANT_LSTM_CMUL":
            return o
    spec = Spec(
        body=(Src0 + Src0 - One) * Src1,
        reference=lambda in0, in1: (2.0 * in0 - 1.0) * in1,
    )
    opcode = dve_ops._CUSTOM_DVE_ROW_BASE + len(dve_ops.OPS)
    shas = {}
    for ver in ("v3", "v4"):
        uops = lower(spec, ver=ver)
        shas[ver] = DveOpSpec(
            name="ANT_LSTM_CMUL", opcode=opcode, uops=uops, rd1_en=has_src1(spec)
        ).sha(ver)
    op = dve_ops.DveOp("ANT_LSTM_CMUL", spec, subdim=False, uops_sha=shas)
    dve_ops.OPS.append(op)
    dve_ops._SUB_OPCODE_FOR_NAME[op.name] = opcode
    return op


def _build(nb=NB, dbg=False):
    import concourse.tile as tile
    from concourse.tile_rust import add_dep_helper
    from concourse import bacc, mybir
    from contextlib import ExitStack

    f32 = mybir.dt.float32
    bf16 = mybir.dt.bfloat16
    AF = mybir.ActivationFunctionType
    CMUL = _register_cmul()

    nc = bacc.Bacc("TRN2", target_bir_lowering=False, debug=False)

    # ---- I/O ----
    x_d = nc.dram_tensor("x", [nb * TS, BS, O], f32, kind="ExternalInput")
    h0_d = nc.dram_tensor("h0T", [H, BS], f32, kind="ExternalInput")
    c0_d = nc.dram_tensor("c0T", [H, BS], f32, kind="ExternalInput")
    wx_d = nc.dram_tensor("Wxp", [sum(KSZ), G4], f32, kind="ExternalInput")
    wh_d = nc.dram_tensor("Whp", [H, G4], f32, kind="ExternalInput")
    eye_d = nc.dram_tensor("eye", [128, 128], f32, kind="ExternalInput")
    kp_d = nc.dram_tensor("kpad", [128, RB], f32, kind="ExternalInput")
    wa1_d = nc.dram_tensor("Wa1", [H, H], f32, kind="ExternalInput")
    ba1_d = nc.dram_tensor("ba1", [H, 1], f32, kind="ExternalInput")
    wa2_d = nc.dram_tensor("Wa2", [H, A], f32, kind="ExternalInput")
    ba2_d = nc.dram_tensor("ba2", [A, 1], f32, kind="ExternalInput")
    wc1_d = nc.dram_tensor("Wc1", [H, H], f32, kind="ExternalInput")
    bc1_d = nc.dram_tensor("bc1", [H, 1], f32, kind="ExternalInput")
    wc2_d = nc.dram_tensor("Wc2", [H, 1], f32, kind="ExternalInput")
    bc2_d = nc.dram_tensor("bc2", [1, 1], f32, kind="ExternalInput")
    ls_d = nc.dram_tensor("logstd", [A, 1], f32, kind="ExternalInput")

    oh_d = nc.dram_tensor("out_h", [H, BS], f32, kind="ExternalOutput")
    oc_d = nc.dram_tensor("out_c", [H, BS], f32, kind="ExternalOutput")
    om_d = nc.dram_tensor("out_m", [A, BS], f32, kind="ExternalOutput")
    ov_d = nc.dram_tensor("out_v", [1, BS], f32, kind="ExternalOutput")
    os_d = nc.dram_tensor("out_s", [A, 1], f32, kind="ExternalOutput")

    if dbg:
        od1_d = nc.dram_tensor("out_sig0", [H, 4 * BG], f32, kind="ExternalOutput")
        od2_d = nc.dram_tensor("out_c1", [H, BS], f32, kind="ExternalOutput")

    with tile.TileContext(nc) as tc, ExitStack() as ctx:
        const = ctx.enter_context(tc.tile_pool(name="const", bufs=1))
        natp = ctx.enter_context(tc.tile_pool(name="nat", bufs=3))
        xtp = ctx.enter_context(tc.tile_pool(name="xt", bufs=1))
        sigp = ctx.enter_context(tc.tile_pool(name="sig", bufs=4))
        vecp = ctx.enter_context(tc.tile_pool(name="vec", bufs=3))
        hbp = ctx.enter_context(tc.tile_pool(name="hb", bufs=3))
        pzp = ctx.enter_context(tc.tile_pool(name="pz", bufs=2, space="PSUM"))
        ptrp = ctx.enter_context(tc.tile_pool(name="ptr", bufs=2, space="PSUM"))
        phdp = ctx.enter_context(tc.tile_pool(name="phd", bufs=1, space="PSUM"))

        dma = nc.sync.dma_start
        cdma = nc.gpsimd.dma_start  # SWDGE: casts f32->bf16 during transfer

        # ---- constants / weights to SBUF ----
        ls_sb = const.tile([A, 1], f32, tag="ls")
        dma(out=ls_sb[:], in_=ls_d.ap())
        std_sb = const.tile([A, 1], f32, tag="std")
        # exp FIRST on ACT (exp table), then everything else uses sigmoid table
        nc.scalar.activation(std_sb[:], ls_sb[:], AF.Exp)
        dma(out=os_d.ap(), in_=std_sb[:])

        eye = const.tile([128, 128], bf16, tag="eye")
        cdma(out=eye[:], in_=eye_d.ap())

        wx_sb = []
        r0 = 0
        for kt in range(KT):
            t_ = const.tile([KSZ[kt], G4], bf16, tag=f"wx{kt}")
            cdma(out=t_[:], in_=wx_d.ap()[r0 : r0 + KSZ[kt], :])
            wx_sb.append(t_)
            r0 += KSZ[kt]

        wh_b = const.tile([H, G4], bf16, tag="whb")
        cdma(out=wh_b[:], in_=wh_d.ap())

        def load_cast(d, shape, tag):
            tb = const.tile(shape, bf16, tag=tag + "b")
            cdma(out=tb[:], in_=d.ap())
            return tb

        wa1_b = load_cast(wa1_d, [H, H], "wa1")
        wa2_b = load_cast(wa2_d, [H, A], "wa2")
        wc1_b = load_cast(wc1_d, [H, H], "wc1")
        wc2_b = load_cast(wc2_d, [H, 1], "wc2")
        ba1_sb = const.tile([H, 1], f32, tag="ba1")
        dma(out=ba1_sb[:], in_=ba1_d.ap())
        ba2_sb = const.tile([A, 1], f32, tag="ba2")
        dma(out=ba2_sb[:], in_=ba2_d.ap())
        bc1_sb = const.tile([H, 1], f32, tag="bc1")
        dma(out=bc1_sb[:], in_=bc1_d.ap())
        bc2_sb = const.tile([1, 1], f32, tag="bc2")
        dma(out=bc2_sb[:], in_=bc2_d.ap())

        # ---- state: c [128, 32] f32 (chain q owns cols 16q:16q+16);
        #      h per-chain bf16 tiles ----
        h0f = const.tile([H, BS], f32, tag="h0f")
        dma(out=h0f[:], in_=h0_d.ap())
        hq = []
        for q in range(GN):
            h0b = const.tile([H, BG], bf16, tag=f"h0b{q}")
            nc.vector.tensor_copy(h0b[:], h0f[:, q * BG : (q + 1) * BG])
            hq.append(h0b)
        cT = const.tile([H, BS], f32, tag="cT")
        dma(out=cT[:], in_=c0_d.ap())

        # xT double buffers; k-tile 5 pad: zeros at 80:96, ones at 96:128 so
        # the bias row at weight-partition 96 contributes b, rest 0
        xt_bufs = []
        for j in range(2):
            xb = xtp.tile([128, KT * RB], bf16, tag=f"xtb{j}")
            cdma(out=xb[:, 5 * RB : 6 * RB], in_=kp_d.ap())
            xt_bufs.append(xb)

        sig_last = [None, None]
        sc_last = [None, None]
        last_sig_a = None

        pz_of = {}

        def prep_gen(blk):
            """Emit block `blk`'s input pipeline (DMA, transposes, PSUM->SBUF
            copies, xW matmuls) in small units; the caller interleaves these
            between recurrence-step emissions so the PE stream has no long
            idle stretches (keeps the HAM clock warm)."""
            t0 = blk * TS
            xt = xt_bufs[blk % 2]
            nats = []
            for r in range(2):
                nat = natp.tile([128, O], bf16, tag="nat")
                src = x_d.ap()[t0 + 4 * r : t0 + 4 * r + 4, :, :]
                cdma(out=nat[:], in_=src.rearrange("a b c -> (a b) c"))
                nats.append(nat)
            yield
            for kt in range(KT):
                cw = 80 if kt == 5 else 128
                ptr = ptrp.tile([128, 256], bf16, tag="ptr")
                for r in range(2):
                    nc.tensor.transpose(
                        ptr[0:cw, 128 * r : 128 * r + 128],
                        nats[r][:, 128 * kt : 128 * kt + cw],
                        eye[:],
                    )
                    yield
                if kt % 2 == 0:
                    nc.vector.tensor_copy(
                        xt[0:cw, kt * RB : kt * RB + RB], ptr[0:cw, :]
                    )
                else:
                    nc.scalar.copy(
                        xt[0:cw, kt * RB : kt * RB + RB], ptr[0:cw, :]
                    )
                yield
            pz = pzp.tile([128, 4 * RB], f32, tag="pz")
            pz_of[blk] = pz
            for g in range(4):
                for kt in range(KT):
                    nc.tensor.matmul(
                        pz[:, g * RB : (g + 1) * RB],
                        wx_sb[kt][:, g * H : (g + 1) * H],
                        xt[0 : KSZ[kt], kt * RB : (kt + 1) * RB],
                        # start=True clears has_written for the WHOLE bank:
                        # issue exactly once per bank (gates 0/2 lead banks 0/1)
                        start=(kt == 0 and g in (0, 2)),
                        stop=False,
                        skip_group_check=True,
                    )
                    yield

        for _ in prep_gen(0):
            pass

        for blk in range(nb):
            pz = pz_of[blk]
            gen = prep_gen(blk + 1) if blk + 1 < nb else None

            # recurrence: 2 phase-shifted half-chains of 16; chain B emitted
            # one step behind chain A so their stages overlap across engines
            steps = []
            for t in range(TS):
                steps.append((t, 0))
                if blk == 0 and t == 0:
                    pass
                elif t == 0:
                    steps.append((TS - 1, 1, blk - 1))
                else:
                    steps.append((t - 1, 1))
            if blk == nb - 1:
                steps.append((TS - 1, 1))
            for st in steps:
                if len(st) == 3:
                    t, q, sblk = st
                    spz = pz_of[blk - 1]
                else:
                    t, q = st
                    spz = pz
                if True:
                    co = 32 * t + BG * q
                    mms = []
                    for g in range(4):
                        mm = nc.tensor.matmul(
                            spz[:, g * RB + co : g * RB + co + BG],
                            wh_b[:, g * H : (g + 1) * H],
                            hq[q][:],
                            start=False,
                            stop=(t == TS - 1),
                            skip_group_check=True,
                        )
                        mms.append(mm)
                    if q == 1 and last_sig_a is not None:
                        add_dep_helper(
                            mms[0].ins, last_sig_a.ins,
                            reason="anti-phase chains",
                        )
                    sig = sigp.tile([128, 4 * BG], bf16, tag=f"sig{q}")
                    zview = spz[:].rearrange("p (g c) -> p g c", g=4)[
                        :, :, co : co + BG
                    ]
                    sview = sig[:].rearrange("p (g c) -> p g c", g=4)
                    sig_i = nc.scalar.activation(sview, zview, AF.Sigmoid)
                    if q == 0:
                        last_sig_a = sig_i
                    s_i = sig[:, 0:BG]
                    s_f = sig[:, BG : 2 * BG]
                    s_o = sig[:, 2 * BG : 3 * BG]
                    s_g = sig[:, 3 * BG : 4 * BG]
                    cq = cT[:, q * BG : (q + 1) * BG]
                    u = vecp.tile([H, BG], f32, tag=f"u{q}")
                    nc.vector._custom_dve(CMUL, out=u[:], in0=s_g, in1=s_i)
                    e = vecp.tile([H, BG], f32, tag=f"e{q}")
                    nc.vector.tensor_mul(e[:], s_f, cq)
                    nc.vector.tensor_add(cq, e[:], u[:])
                    sc = vecp.tile([H, BG], bf16, tag=f"sc{q}")
                    nc.scalar.activation(sc[:], cq, AF.Sigmoid, scale=2.0)
                    hn = hbp.tile([H, BG], bf16, tag=f"h{q}")
                    nc.vector._custom_dve(CMUL, out=hn[:], in0=sc[:], in1=s_o)
                    hq[q] = hn
                    if blk == nb - 1 and t == TS - 1:
                        sig_last[q] = sig
                        sc_last[q] = sc
                if gen is not None:
                    n_units = 2 if (t + q) % 2 == 0 else 3
                    for _ in range(n_units):
                        if next(gen, "done") == "done":
                            gen = None
                            break
            while gen is not None and next(gen, "done") != "done":
                pass

        # ---- outputs ----
        hf = const.tile([H, BS], f32, tag="hf")
        hb_all = const.tile([H, BS], bf16, tag="hball")
        for q in range(GN):
            nc.vector._custom_dve(
                CMUL,
                out=hf[:, q * BG : (q + 1) * BG],
                in0=sc_last[q][:],
                in1=sig_last[q][:, 2 * BG : 3 * BG],
            )
            nc.vector.tensor_copy(hb_all[:, q * BG : (q + 1) * BG], hq[q][:])
        dma(out=oh_d.ap(), in_=hf[:])
        dma(out=oc_d.ap(), in_=cT[:])

        # actor head
        p1 = phdp.tile([H, BS], f32, tag="ph")
        nc.tensor.matmul(p1[:], wa1_b[:], hb_all[:], start=True, stop=True)
        a1 = const.tile([H, BS], bf16, tag="a1")
        nc.scalar.activation(a1[:], p1[:], AF.Tanh, bias=ba1_sb[:, 0:1])
        p2 = phdp.tile([A, BS], f32, tag="ph2")
        nc.tensor.matmul(p2[:], wa2_b[:], a1[:], start=True, stop=True)
        m_sb = const.tile([A, BS], f32, tag="msb")
        nc.scalar.activation(m_sb[:], p2[:], AF.Identity, bias=ba2_sb[:, 0:1])
        dma(out=om_d.ap(), in_=m_sb[:])

        # critic head
        q1 = phdp.tile([H, BS], f32, tag="ph")
        nc.tensor.matmul(q1[:], wc1_b[:], hb_all[:], start=True, stop=True)
        c1 = const.tile([H, BS], bf16, tag="c1")
        nc.scalar.activation(c1[:], q1[:], AF.Tanh, bias=bc1_sb[:, 0:1])
        q2 = phdp.tile([1, BS], f32, tag="ph2")
        nc.tensor.matmul(q2[:], wc2_b[:], c1[:], start=True, stop=True)
        v_sb = const.tile([1, BS], f32, tag="vsb")
        nc.scalar.activation(v_sb[:], q2[:], AF.Identity, bias=bc2_sb[:, 0:1])
        dma(out=ov_d.ap(), in_=v_sb[:])

    nc.compile()
    return nc


def kernel(x, h0, c0, Wx, Wh, b, Wa1, ba1, Wa2, ba2, log_std, Wc1, bc1, Wc2, bc2):
    from concourse.bass_utils import run_bass_kernel_spmd

    x = np.asarray(x, np.float32)
    h0 = np.asarray(h0, np.float32)
    c0 = np.asarray(c0, np.float32)
    Wx = np.asarray(Wx, np.float32)
    Wh = np.asarray(Wh, np.float32)
    b = np.asarray(b, np.float32)

    # gate order in reference: [i, f, g, o]; ours: [i, f, o, g] with g scaled x2
    perm = np.concatenate(
        [np.arange(0, H), np.arange(H, 2 * H), np.arange(3 * H, 4 * H),
         np.arange(2 * H, 3 * H)]
    )
    Wxp = np.vstack(
        [Wx, np.zeros((16, 4 * H), np.float32), b[None, :],
         np.zeros((31, 4 * H), np.float32)]
    )[:, perm].copy()
    Wxp[:, 3 * H :] *= 2.0
    Whp = Wh[:, perm].copy()
    Whp[:, 3 * H :] *= 2.0

    if "nc" not in _nc_cache:
        _nc_cache["nc"] = _build()
    nc = _nc_cache["nc"]

    common = {
        "Wxp": np.ascontiguousarray(Wxp),
        "Whp": np.ascontiguousarray(Whp),
        "eye": np.eye(128, dtype=np.float32),
        "kpad": np.vstack([np.zeros((96, RB), np.float32),
                           np.ones((32, RB), np.float32)]),
        "Wa1": np.asarray(Wa1, np.float32),
        "ba1": np.asarray(ba1, np.float32).reshape(H, 1),
        "Wa2": np.asarray(Wa2, np.float32),
        "ba2": np.asarray(ba2, np.float32).reshape(A, 1),
        "Wc1": np.asarray(Wc1, np.float32),
        "bc1": np.asarray(bc1, np.float32).reshape(H, 1),
        "Wc2": np.asarray(Wc2, np.float32),
        "bc2": np.asarray(bc2, np.float32).reshape(1, 1),
        "logstd": np.asarray(log_std, np.float32).reshape(A, 1),
    }
    in_maps = []
    for c in range(NCORES):
        bs = slice(c * BS, (c + 1) * BS)
        m = dict(common)
        m["x"] = np.ascontiguousarray(x[:, bs, :])
        m["h0T"] = np.ascontiguousarray(h0[bs].T)
        m["c0T"] = np.ascontiguousarray(c0[bs].T)
        in_maps.append(m)

    res = run_bass_kernel_spmd(nc, in_maps, core_ids=list(range(NCORES)))
    rs = res.results

    hT_full = np.concatenate([rs[c]["out_h"].T for c in range(NCORES)], 0)
    cT_full = np.concatenate([rs[c]["out_c"].T for c in range(NCORES)], 0)
    mean = np.concatenate([rs[c]["out_m"].T for c in range(NCORES)], 0)
    value = np.concatenate([rs[c]["out_v"][0] for c in range(NCORES)], 0)
    std = rs[0]["out_s"][:, 0]
    return mean, std, value, hT_full, cT_full


# revision 28
# speedup vs baseline: 1.0556x; 1.0551x over previous
"""Trainium2 Bass kernel: BasicLSTMActorCritic, data-parallel over batch on 8 cores.

Per-core shard: B=32 of 256.  T=512, B=256, O=720, H=128, A=2.
Design:
  - Host prep: gate columns permuted to [i, f, o, g]; g-gate weights scaled x2 so
    tanh(z) = 2*sigmoid(2z)-1 means ONE sigmoid covers all 4 gates.  Bias b
    folded into Wx as an extra contraction row (ones row in x^T on chip).
  - All-bf16 x pipeline: x DMA-cast f32->bf16 on load (SWDGE), PE-transposed
    to x^T, bf16 xW matmuls accumulate z^T directly in PSUM (f32).
  - Recurrence: 512 serial steps, batch split into 2 phase-shifted half-chains
    of 16 so the two chains' stages overlap across engines.  Per chain-step:
    4 bf16 matmuls (Wh stationary, h moving) accumulate onto the PSUM z,
    one sigmoid (ACT) for all 4 gates, then on DVE: u = CMUL(sg, si) =
    (2*sg-1)*si = tanh(zg)*si, e = sf*c, c = e+u, and h = CMUL(sigmoid(2c), so)
    = tanh(c)*so — the custom CMUL op removes the ACT tanh entirely.
  - PSUM->SBUF x^T copies ride DMA (SP HWDGE), not DVE.
  - Heads (actor/critic MLPs) computed once at the end from final h.
"""

import sys

sys.path.insert(0, "/opt/trn_rl_repo")

import numpy as np

T, B, O, H, A = 512, 256, 720, 128, 2
NCORES = 8
BS = B // NCORES  # 32 batch per core
GN = 2  # phase-shifted half-chains
BG = BS // GN  # 16 batch per chain
G4 = 4 * H  # 512
TS = 8  # timesteps per block
NB = T // TS  # 64 blocks
RB = TS * BS  # 256 moving cols per block
KT = 6  # K tiles over the padded contraction dim
# last k-tile rows: [80 x rows; 16 zero rows; bias row at 96; 31 zero rows] —
# engine APs need base partition 0 (any count) or 32/64/96 (count <= 32)
KSZ = [128, 128, 128, 128, 128, 128]

_nc_cache = {}


def _register_cmul():
    from concourse import dve_ops
    from concourse.dve_spec import Spec, Src0, Src1, One, lower
    from concourse.dve_spec import _has_src1 as has_src1
    from concourse.dve_uop import DveOpSpec

    for o in dve_ops.OPS:
        if o.name == "ANT_LSTM_CMUL":
            return o
    spec = Spec(
        body=(Src0 + Src0 - One) * Src1,
        reference=lambda in0, in1: (2.0 * in0 - 1.0) * in1,
    )
    opcode = dve_ops._CUSTOM_DVE_ROW_BASE + len(dve_ops.OPS)
    shas = {}
    for ver in ("v3", "v4"):
        uops = lower(spec, ver=ver)
        shas[ver] = DveOpSpec(
            name="ANT_LSTM_CMUL", opcode=opcode, uops=uops, rd1_en=has_src1(spec)
        ).sha(ver)
    op = dve_ops.DveOp("ANT_LSTM_CMUL", spec, subdim=False, uops_sha=shas)
    dve_ops.OPS.append(op)
    dve_ops._SUB_OPCODE_FOR_NAME[op.name] = opcode
    return op


def _build(nb=NB, dbg=False):
    import concourse.tile as tile
    from concourse.tile_rust import add_dep_helper
    from concourse import bacc, mybir
    from contextlib import ExitStack

    f32 = mybir.dt.float32
    bf16 = mybir.dt.bfloat16
    AF = mybir.ActivationFunctionType
    CMUL = _register_cmul()

    nc = bacc.Bacc("TRN2", target_bir_lowering=False, debug=False)

    # ---- I/O ----
    x_d = nc.dram_tensor("x", [nb * TS, BS, O], f32, kind="ExternalInput")
    h0_d = nc.dram_tensor("h0T", [H, BS], f32, kind="ExternalInput")
    c0_d = nc.dram_tensor("c0T", [H, BS], f32, kind="ExternalInput")
    wx_d = nc.dram_tensor("Wxp", [sum(KSZ), G4], f32, kind="ExternalInput")
    wh_d = nc.dram_tensor("Whp", [H, G4], f32, kind="ExternalInput")
    eye_d = nc.dram_tensor("eye", [128, 128], f32, kind="ExternalInput")
    kp_d = nc.dram_tensor("kpad", [128, RB], f32, kind="ExternalInput")
    wa1_d = nc.dram_tensor("Wa1", [H, H], f32, kind="ExternalInput")
    ba1_d = nc.dram_tensor("ba1", [H, 1], f32, kind="ExternalInput")
    wa2_d = nc.dram_tensor("Wa2", [H, A], f32, kind="ExternalInput")
    ba2_d = nc.dram_tensor("ba2", [A, 1], f32, kind="ExternalInput")
    wc1_d = nc.dram_tensor("Wc1", [H, H], f32, kind="ExternalInput")
    bc1_d = nc.dram_tensor("bc1", [H, 1], f32, kind="ExternalInput")
    wc2_d = nc.dram_tensor("Wc2", [H, 1], f32, kind="ExternalInput")
    bc2_d = nc.dram_tensor("bc2", [1, 1], f32, kind="ExternalInput")
    ls_d = nc.dram_tensor("logstd", [A, 1], f32, kind="ExternalInput")

    oh_d = nc.dram_tensor("out_h", [H, BS], f32, kind="ExternalOutput")
    oc_d = nc.dram_tensor("out_c", [H, BS], f32, kind="ExternalOutput")
    om_d = nc.dram_tensor("out_m", [A, BS], f32, kind="ExternalOutput")
    ov_d = nc.dram_tensor("out_v", [1, BS], f32, kind="ExternalOutput")
    os_d = nc.dram_tensor("out_s", [A, 1], f32, kind="ExternalOutput")

    if dbg:
        od1_d = nc.dram_tensor("out_sig0", [H, 4 * BG], f32, kind="ExternalOutput")
        od2_d = nc.dram_tensor("out_c1", [H, BS], f32, kind="ExternalOutput")

    with tile.TileContext(nc) as tc, ExitStack() as ctx:
        const = ctx.enter_context(tc.tile_pool(name="const", bufs=1))
        natp = ctx.enter_context(tc.tile_pool(name="nat", bufs=3))
        xtp = ctx.enter_context(tc.tile_pool(name="xt", bufs=1))
        sigp = ctx.enter_context(tc.tile_pool(name="sig", bufs=4))
        vecp = ctx.enter_context(tc.tile_pool(name="vec", bufs=3))
        hbp = ctx.enter_context(tc.tile_pool(name="hb", bufs=3))
        pzp = ctx.enter_context(tc.tile_pool(name="pz", bufs=2, space="PSUM"))
        ptrp = ctx.enter_context(tc.tile_pool(name="ptr", bufs=2, space="PSUM"))
        phdp = ctx.enter_context(tc.tile_pool(name="phd", bufs=1, space="PSUM"))

        dma = nc.sync.dma_start
        cdma = nc.gpsimd.dma_start  # SWDGE: casts f32->bf16 during transfer

        # ---- constants / weights to SBUF ----
        ls_sb = const.tile([A, 1], f32, tag="ls")
        dma(out=ls_sb[:], in_=ls_d.ap())
        std_sb = const.tile([A, 1], f32, tag="std")
        # exp FIRST on ACT (exp table), then everything else uses sigmoid table
        nc.scalar.activation(std_sb[:], ls_sb[:], AF.Exp)
        dma(out=os_d.ap(), in_=std_sb[:])

        eye = const.tile([128, 128], bf16, tag="eye")
        cdma(out=eye[:], in_=eye_d.ap())

        wx_sb = []
        r0 = 0
        for kt in range(KT):
            t_ = const.tile([KSZ[kt], G4], bf16, tag=f"wx{kt}")
            cdma(out=t_[:], in_=wx_d.ap()[r0 : r0 + KSZ[kt], :])
            wx_sb.append(t_)
            r0 += KSZ[kt]

        wh_b = const.tile([H, G4], bf16, tag="whb")
        cdma(out=wh_b[:], in_=wh_d.ap())

        def load_cast(d, shape, tag):
            tb = const.tile(shape, bf16, tag=tag + "b")
            cdma(out=tb[:], in_=d.ap())
            return tb

        wa1_b = load_cast(wa1_d, [H, H], "wa1")
        wa2_b = load_cast(wa2_d, [H, A], "wa2")
        wc1_b = load_cast(wc1_d, [H, H], "wc1")
        wc2_b = load_cast(wc2_d, [H, 1], "wc2")
        ba1_sb = const.tile([H, 1], f32, tag="ba1")
        dma(out=ba1_sb[:], in_=ba1_d.ap())
        ba2_sb = const.tile([A, 1], f32, tag="ba2")
        dma(out=ba2_sb[:], in_=ba2_d.ap())
        bc1_sb = const.tile([H, 1], f32, tag="bc1")
        dma(out=bc1_sb[:], in_=bc1_d.ap())
        bc2_sb = const.tile([1, 1], f32, tag="bc2")
        dma(out=bc2_sb[:], in_=bc2_d.ap())

        # ---- state: c [128, 32] f32 (chain q owns cols 16q:16q+16);
        #      h per-chain bf16 tiles ----
        h0f = const.tile([H, BS], f32, tag="h0f")
        dma(out=h0f[:], in_=h0_d.ap())
        hq = []
        for q in range(GN):
            h0b = const.tile([H, BG], bf16, tag=f"h0b{q}")
            nc.vector.tensor_copy(h0b[:], h0f[:, q * BG : (q + 1) * BG])
            hq.append(h0b)
        cq_t = []
        for q in range(GN):
            cqt = const.tile([H, BG], f32, tag=f"cT{q}")
            dma(out=cqt[:], in_=c0_d.ap()[:, q * BG : (q + 1) * BG])
            cq_t.append(cqt)

        # xT double buffers; k-tile 5 pad: zeros at 80:96, ones at 96:128 so
        # the bias row at weight-partition 96 contributes b, rest 0
        xt_bufs = []
        for j in range(2):
            xb = xtp.tile([128, KT * RB], bf16, tag=f"xtb{j}")
            cdma(out=xb[:, 5 * RB : 6 * RB], in_=kp_d.ap())
            xt_bufs.append(xb)

        sig_last = [None, None]
        sc_last = [None, None]
        last_sig_a = None

        pz_of = {}

        def prep_gen(blk):
            """Emit block `blk`'s input pipeline (DMA, transposes, PSUM->SBUF
            copies, xW matmuls) in small units; the caller interleaves these
            between recurrence-step emissions so the PE stream has no long
            idle stretches (keeps the HAM clock warm)."""
            t0 = blk * TS
            xt = xt_bufs[blk % 2]
            nats = []
            for r in range(2):
                nat = natp.tile([128, O], bf16, tag="nat")
                src = x_d.ap()[t0 + 4 * r : t0 + 4 * r + 4, :, :]
                cdma(out=nat[:], in_=src.rearrange("a b c -> (a b) c"))
                nats.append(nat)
            yield
            for kt in range(KT):
                cw = 80 if kt == 5 else 128
                ptr = ptrp.tile([128, 256], bf16, tag="ptr")
                for r in range(2):
                    nc.tensor.transpose(
                        ptr[0:cw, 128 * r : 128 * r + 128],
                        nats[r][:, 128 * kt : 128 * kt + cw],
                        eye[:],
                    )
                    yield
                if kt % 2 == 0:
                    nc.vector.tensor_copy(
                        xt[0:cw, kt * RB : kt * RB + RB], ptr[0:cw, :]
                    )
                else:
                    nc.scalar.copy(
                        xt[0:cw, kt * RB : kt * RB + RB], ptr[0:cw, :]
                    )
                yield
            # one PSUM bank per chain so the bank-overlap tracker never
            # serializes chain A's sigma reads against chain B's matmul
            # writes; layout per bank: gate-major, [g*128 + 16*t + b]
            pzq = []
            for q in range(GN):
                pzt = pzp.tile([128, 4 * TS * BG], f32, tag=f"pz{q}")
                pzq.append(pzt)
            pz_of[blk] = pzq
            GW = TS * BG  # 128 cols per gate region
            for q in range(GN):
                for g in range(4):
                    for kt in range(KT):
                        rview = xt[0 : KSZ[kt], kt * RB : (kt + 1) * RB]
                        rview = rview.rearrange(
                            "p (t c) -> p t c", t=TS
                        )[:, :, q * BG : (q + 1) * BG]
                        nc.tensor.matmul(
                            pzq[q][:, g * GW : (g + 1) * GW],
                            wx_sb[kt][:, g * H : (g + 1) * H],
                            rview,
                            # start=True clears has_written for the WHOLE
                            # bank: exactly once per chain-bank
                            start=(kt == 0 and g == 0),
                            stop=False,
                            skip_group_check=True,
                        )
                        yield

        for _ in prep_gen(0):
            pass

        GW = TS * BG
        for blk in range(nb):
            pzq = pz_of[blk]
            gen = prep_gen(blk + 1) if blk + 1 < nb else None

            # recurrence: 2 phase-shifted half-chains of 16; chain B emitted
            # one step behind chain A so their stages overlap across engines
            steps = []
            for t in range(TS):
                steps.append((t, 0))
                if blk == 0 and t == 0:
                    pass
                elif t == 0:
                    steps.append((TS - 1, 1, blk - 1))
                else:
                    steps.append((t - 1, 1))
            if blk == nb - 1:
                steps.append((TS - 1, 1))
            for st in steps:
                if len(st) == 3:
                    t, q, sblk = st
                    spz = pz_of[blk - 1][q]
                else:
                    t, q = st
                    spz = pzq[q]
                if True:
                    co = BG * t
                    mms = []
                    for g in range(4):
                        mm = nc.tensor.matmul(
                            spz[:, g * GW + co : g * GW + co + BG],
                            wh_b[:, g * H : (g + 1) * H],
                            hq[q][:],
                            start=False,
                            stop=(t == TS - 1),
                            skip_group_check=True,
                        )
                        mms.append(mm)
                    if q == 1 and last_sig_a is not None:
                        add_dep_helper(
                            mms[0].ins, last_sig_a.ins,
                            reason="anti-phase chains",
                        )
                    sig = sigp.tile([128, 4 * BG], bf16, tag=f"sig{q}")
                    zview = spz[:].rearrange("p (g c) -> p g c", g=4)[
                        :, :, co : co + BG
                    ]
                    sview = sig[:].rearrange("p (g c) -> p g c", g=4)
                    sig_i = nc.scalar.activation(sview, zview, AF.Sigmoid)
                    if q == 0:
                        last_sig_a = sig_i
                    s_i = sig[:, 0:BG]
                    s_f = sig[:, BG : 2 * BG]
                    s_o = sig[:, 2 * BG : 3 * BG]
                    s_g = sig[:, 3 * BG : 4 * BG]
                    cq = cq_t[q][:]
                    u = vecp.tile([H, BG], f32, tag=f"u{q}")
                    nc.vector._custom_dve(CMUL, out=u[:], in0=s_g, in1=s_i)
                    e = vecp.tile([H, BG], f32, tag=f"e{q}")
                    nc.vector.tensor_mul(e[:], s_f, cq)
                    nc.vector.tensor_add(cq, e[:], u[:])
                    sc = vecp.tile([H, BG], bf16, tag=f"sc{q}")
                    nc.scalar.activation(sc[:], cq, AF.Sigmoid, scale=2.0)
                    hn = hbp.tile([H, BG], bf16, tag=f"h{q}")
                    nc.vector._custom_dve(CMUL, out=hn[:], in0=sc[:], in1=s_o)
                    hq[q] = hn
                    if blk == nb - 1 and t == TS - 1:
                        sig_last[q] = sig
                        sc_last[q] = sc
                if gen is not None:
                    n_units = 2 if (t + q) % 2 == 0 else 3
                    for _ in range(n_units):
                        if next(gen, "done") == "done":
                            gen = None
                            break
            while gen is not None and next(gen, "done") != "done":
                pass

        # ---- outputs ----
        hf = const.tile([H, BS], f32, tag="hf")
        hb_all = const.tile([H, BS], bf16, tag="hball")
        for q in range(GN):
            nc.vector._custom_dve(
                CMUL,
                out=hf[:, q * BG : (q + 1) * BG],
                in0=sc_last[q][:],
                in1=sig_last[q][:, 2 * BG : 3 * BG],
            )
            nc.vector.tensor_copy(hb_all[:, q * BG : (q + 1) * BG], hq[q][:])
        dma(out=oh_d.ap(), in_=hf[:])
        for q in range(GN):
            dma(out=oc_d.ap()[:, q * BG : (q + 1) * BG], in_=cq_t[q][:])

        # actor head
        p1 = phdp.tile([H, BS], f32, tag="ph")
        nc.tensor.matmul(p1[:], wa1_b[:], hb_all[:], start=True, stop=True)
        a1 = const.tile([H, BS], bf16, tag="a1")
        nc.scalar.activation(a1[:], p1[:], AF.Tanh, bias=ba1_sb[:, 0:1])
        p2 = phdp.tile([A, BS], f32, tag="ph2")
        nc.tensor.matmul(p2[:], wa2_b[:], a1[:], start=True, stop=True)
        m_sb = const.tile([A, BS], f32, tag="msb")
        nc.scalar.activation(m_sb[:], p2[:], AF.Identity, bias=ba2_sb[:, 0:1])
        dma(out=om_d.ap(), in_=m_sb[:])

        # critic head
        q1 = phdp.tile([H, BS], f32, tag="ph")
        nc.tensor.matmul(q1[:], wc1_b[:], hb_all[:], start=True, stop=True)
        c1 = const.tile([H, BS], bf16, tag="c1")
        nc.scalar.activation(c1[:], q1[:], AF.Tanh, bias=bc1_sb[:, 0:1])
        q2 = phdp.tile([1, BS], f32, tag="ph2")
        nc.tensor.matmul(q2[:], wc2_b[:], c1[:], start=True, stop=True)
        v_sb = const.tile([1, BS], f32, tag="vsb")
        nc.scalar.activation(v_sb[:], q2[:], AF.Identity, bias=bc2_sb[:, 0:1])
        dma(out=ov_d.ap(), in_=v_sb[:])

    nc.compile()
    return nc


def kernel(x, h0, c0, Wx, Wh, b, Wa1, ba1, Wa2, ba2, log_std, Wc1, bc1, Wc2, bc2):
    from concourse.bass_utils import run_bass_kernel_spmd

    x = np.asarray(x, np.float32)
    h0 = np.asarray(h0, np.float32)
    c0 = np.asarray(c0, np.float32)
    Wx = np.asarray(Wx, np.float32)
    Wh = np.asarray(Wh, np.float32)
    b = np.asarray(b, np.float32)

    # gate order in reference: [i, f, g, o]; ours: [i, f, o, g] with g scaled x2
    perm = np.concatenate(
        [np.arange(0, H), np.arange(H, 2 * H), np.arange(3 * H, 4 * H),
         np.arange(2 * H, 3 * H)]
    )
    Wxp = np.vstack(
        [Wx, np.zeros((16, 4 * H), np.float32), b[None, :],
         np.zeros((31, 4 * H), np.float32)]
    )[:, perm].copy()
    Wxp[:, 3 * H :] *= 2.0
    Whp = Wh[:, perm].copy()
    Whp[:, 3 * H :] *= 2.0

    if "nc" not in _nc_cache:
        _nc_cache["nc"] = _build()
    nc = _nc_cache["nc"]

    common = {
        "Wxp": np.ascontiguousarray(Wxp),
        "Whp": np.ascontiguousarray(Whp),
        "eye": np.eye(128, dtype=np.float32),
        "kpad": np.vstack([np.zeros((96, RB), np.float32),
                           np.ones((32, RB), np.float32)]),
        "Wa1": np.asarray(Wa1, np.float32),
        "ba1": np.asarray(ba1, np.float32).reshape(H, 1),
        "Wa2": np.asarray(Wa2, np.float32),
        "ba2": np.asarray(ba2, np.float32).reshape(A, 1),
        "Wc1": np.asarray(Wc1, np.float32),
        "bc1": np.asarray(bc1, np.float32).reshape(H, 1),
        "Wc2": np.asarray(Wc2, np.float32),
        "bc2": np.asarray(bc2, np.float32).reshape(1, 1),
        "logstd": np.asarray(log_std, np.float32).reshape(A, 1),
    }
    in_maps = []
    for c in range(NCORES):
        bs = slice(c * BS, (c + 1) * BS)
        m = dict(common)
        m["x"] = np.ascontiguousarray(x[:, bs, :])
        m["h0T"] = np.ascontiguousarray(h0[bs].T)
        m["c0T"] = np.ascontiguousarray(c0[bs].T)
        in_maps.append(m)

    res = run_bass_kernel_spmd(nc, in_maps, core_ids=list(range(NCORES)))
    rs = res.results

    hT_full = np.concatenate([rs[c]["out_h"].T for c in range(NCORES)], 0)
    cT_full = np.concatenate([rs[c]["out_c"].T for c in range(NCORES)], 0)
    mean = np.concatenate([rs[c]["out_m"].T for c in range(NCORES)], 0)
    value = np.concatenate([rs[c]["out_v"][0] for c in range(NCORES)], 0)
    std = rs[0]["out_s"][:, 0]
    return mean, std, value, hT_full, cT_full


# revision 29
# speedup vs baseline: 1.0662x; 1.0100x over previous
"""Trainium2 Bass kernel: BasicLSTMActorCritic, data-parallel over batch on 8 cores.

Per-core shard: B=32 of 256.  T=512, B=256, O=720, H=128, A=2.
Design:
  - Host prep: gate columns permuted to [i, f, o, g]; g-gate weights scaled x2 so
    tanh(z) = 2*sigmoid(2z)-1 means ONE sigmoid covers all 4 gates.  Bias b
    folded into Wx as an extra contraction row (ones row in x^T on chip).
  - All-bf16 x pipeline: x DMA-cast f32->bf16 on load (SWDGE), PE-transposed
    to x^T, bf16 xW matmuls accumulate z^T directly in PSUM (f32).
  - Recurrence: 512 serial steps, batch split into 2 phase-shifted half-chains
    of 16 so the two chains' stages overlap across engines.  Per chain-step:
    4 bf16 matmuls (Wh stationary, h moving) accumulate onto the PSUM z,
    one sigmoid (ACT) for all 4 gates, then on DVE: u = CMUL(sg, si) =
    (2*sg-1)*si = tanh(zg)*si, e = sf*c, c = e+u, and h = CMUL(sigmoid(2c), so)
    = tanh(c)*so — the custom CMUL op removes the ACT tanh entirely.
  - PSUM->SBUF x^T copies ride DMA (SP HWDGE), not DVE.
  - Heads (actor/critic MLPs) computed once at the end from final h.
"""

import sys

sys.path.insert(0, "/opt/trn_rl_repo")

import numpy as np

T, B, O, H, A = 512, 256, 720, 128, 2
NCORES = 8
BS = B // NCORES  # 32 batch per core
GN = 2  # phase-shifted half-chains
BG = BS // GN  # 16 batch per chain
G4 = 4 * H  # 512
TS = 8  # timesteps per block
NB = T // TS  # 64 blocks
RB = TS * BS  # 256 moving cols per block
KT = 6  # K tiles over the padded contraction dim
# last k-tile rows: [80 x rows; 16 zero rows; bias row at 96; 31 zero rows] —
# engine APs need base partition 0 (any count) or 32/64/96 (count <= 32)
KSZ = [128, 128, 128, 128, 128, 128]

_nc_cache = {}


def _register_cmul():
    from concourse import dve_ops
    from concourse.dve_spec import Spec, Src0, Src1, One, lower
    from concourse.dve_spec import _has_src1 as has_src1
    from concourse.dve_uop import DveOpSpec

    for o in dve_ops.OPS:
        if o.name == "ANT_LSTM_CMUL":
            return o
    spec = Spec(
        body=(Src0 + Src0 - One) * Src1,
        reference=lambda in0, in1: (2.0 * in0 - 1.0) * in1,
    )
    opcode = dve_ops._CUSTOM_DVE_ROW_BASE + len(dve_ops.OPS)
    shas = {}
    for ver in ("v3", "v4"):
        uops = lower(spec, ver=ver)
        shas[ver] = DveOpSpec(
            name="ANT_LSTM_CMUL", opcode=opcode, uops=uops, rd1_en=has_src1(spec)
        ).sha(ver)
    op = dve_ops.DveOp("ANT_LSTM_CMUL", spec, subdim=False, uops_sha=shas)
    dve_ops.OPS.append(op)
    dve_ops._SUB_OPCODE_FOR_NAME[op.name] = opcode
    return op


def _build(nb=NB, dbg=False):
    import concourse.tile as tile
    from concourse.tile_rust import add_dep_helper
    from concourse import bacc, mybir
    from contextlib import ExitStack

    f32 = mybir.dt.float32
    bf16 = mybir.dt.bfloat16
    AF = mybir.ActivationFunctionType
    CMUL = _register_cmul()

    nc = bacc.Bacc("TRN2", target_bir_lowering=False, debug=False)

    # ---- I/O ----
    x_d = nc.dram_tensor("x", [nb * TS, BS, O], f32, kind="ExternalInput")
    h0_d = nc.dram_tensor("h0T", [H, BS], f32, kind="ExternalInput")
    c0_d = nc.dram_tensor("c0T", [H, BS], f32, kind="ExternalInput")
    wx_d = nc.dram_tensor("Wxp", [sum(KSZ), G4], f32, kind="ExternalInput")
    wh_d = nc.dram_tensor("Whp", [H, G4], f32, kind="ExternalInput")
    eye_d = nc.dram_tensor("eye", [128, 128], f32, kind="ExternalInput")
    kp_d = nc.dram_tensor("kpad", [128, RB], f32, kind="ExternalInput")
    wa1_d = nc.dram_tensor("Wa1", [H, H], f32, kind="ExternalInput")
    ba1_d = nc.dram_tensor("ba1", [H, 1], f32, kind="ExternalInput")
    wa2_d = nc.dram_tensor("Wa2", [H, A], f32, kind="ExternalInput")
    ba2_d = nc.dram_tensor("ba2", [A, 1], f32, kind="ExternalInput")
    wc1_d = nc.dram_tensor("Wc1", [H, H], f32, kind="ExternalInput")
    bc1_d = nc.dram_tensor("bc1", [H, 1], f32, kind="ExternalInput")
    wc2_d = nc.dram_tensor("Wc2", [H, 1], f32, kind="ExternalInput")
    bc2_d = nc.dram_tensor("bc2", [1, 1], f32, kind="ExternalInput")
    ls_d = nc.dram_tensor("logstd", [A, 1], f32, kind="ExternalInput")

    oh_d = nc.dram_tensor("out_h", [H, BS], f32, kind="ExternalOutput")
    oc_d = nc.dram_tensor("out_c", [H, BS], f32, kind="ExternalOutput")
    om_d = nc.dram_tensor("out_m", [A, BS], f32, kind="ExternalOutput")
    ov_d = nc.dram_tensor("out_v", [1, BS], f32, kind="ExternalOutput")
    os_d = nc.dram_tensor("out_s", [A, 1], f32, kind="ExternalOutput")

    if dbg:
        od1_d = nc.dram_tensor("out_sig0", [H, 4 * BG], f32, kind="ExternalOutput")
        od2_d = nc.dram_tensor("out_c1", [H, BS], f32, kind="ExternalOutput")

    with tile.TileContext(nc) as tc, ExitStack() as ctx:
        const = ctx.enter_context(tc.tile_pool(name="const", bufs=1))
        natp = ctx.enter_context(tc.tile_pool(name="nat", bufs=3))
        xtp = ctx.enter_context(tc.tile_pool(name="xt", bufs=1))
        sigp = ctx.enter_context(tc.tile_pool(name="sig", bufs=4))
        vecp = ctx.enter_context(tc.tile_pool(name="vec", bufs=3))
        hbp = ctx.enter_context(tc.tile_pool(name="hb", bufs=3))
        pzp = ctx.enter_context(tc.tile_pool(name="pz", bufs=2, space="PSUM"))
        ptrp = ctx.enter_context(tc.tile_pool(name="ptr", bufs=2, space="PSUM"))
        phdp = ctx.enter_context(tc.tile_pool(name="phd", bufs=1, space="PSUM"))

        dma = nc.sync.dma_start
        cdma = nc.gpsimd.dma_start  # SWDGE: casts f32->bf16 during transfer

        # ---- constants / weights to SBUF ----
        ls_sb = const.tile([A, 1], f32, tag="ls")
        dma(out=ls_sb[:], in_=ls_d.ap())
        std_sb = const.tile([A, 1], f32, tag="std")
        # exp FIRST on ACT (exp table), then everything else uses sigmoid table
        nc.scalar.activation(std_sb[:], ls_sb[:], AF.Exp)
        dma(out=os_d.ap(), in_=std_sb[:])

        eye = const.tile([128, 128], bf16, tag="eye")
        cdma(out=eye[:], in_=eye_d.ap())

        wx_sb = []
        r0 = 0
        for kt in range(KT):
            t_ = const.tile([KSZ[kt], G4], bf16, tag=f"wx{kt}")
            cdma(out=t_[:], in_=wx_d.ap()[r0 : r0 + KSZ[kt], :])
            wx_sb.append(t_)
            r0 += KSZ[kt]

        wh_b = const.tile([H, G4], bf16, tag="whb")
        cdma(out=wh_b[:], in_=wh_d.ap())

        def load_cast(d, shape, tag):
            tb = const.tile(shape, bf16, tag=tag + "b")
            cdma(out=tb[:], in_=d.ap())
            return tb

        wa1_b = load_cast(wa1_d, [H, H], "wa1")
        wa2_b = load_cast(wa2_d, [H, A], "wa2")
        wc1_b = load_cast(wc1_d, [H, H], "wc1")
        wc2_b = load_cast(wc2_d, [H, 1], "wc2")
        ba1_sb = const.tile([H, 1], f32, tag="ba1")
        dma(out=ba1_sb[:], in_=ba1_d.ap())
        ba2_sb = const.tile([A, 1], f32, tag="ba2")
        dma(out=ba2_sb[:], in_=ba2_d.ap())
        bc1_sb = const.tile([H, 1], f32, tag="bc1")
        dma(out=bc1_sb[:], in_=bc1_d.ap())
        bc2_sb = const.tile([1, 1], f32, tag="bc2")
        dma(out=bc2_sb[:], in_=bc2_d.ap())

        # ---- state: c [128, 32] f32 (chain q owns cols 16q:16q+16);
        #      h per-chain bf16 tiles ----
        h0f = const.tile([H, BS], f32, tag="h0f")
        dma(out=h0f[:], in_=h0_d.ap())
        hq = []
        for q in range(GN):
            h0b = const.tile([H, BG], bf16, tag=f"h0b{q}")
            nc.vector.tensor_copy(h0b[:], h0f[:, q * BG : (q + 1) * BG])
            hq.append(h0b)
        cq_t = []
        for q in range(GN):
            cqt = const.tile([H, BG], f32, tag=f"cT{q}")
            dma(out=cqt[:], in_=c0_d.ap()[:, q * BG : (q + 1) * BG])
            cq_t.append(cqt)

        # xT double buffers; k-tile 5 pad: zeros at 80:96, ones at 96:128 so
        # the bias row at weight-partition 96 contributes b, rest 0
        xt_bufs = []
        for j in range(2):
            xb = xtp.tile([128, KT * RB], bf16, tag=f"xtb{j}")
            cdma(out=xb[:, 5 * RB : 6 * RB], in_=kp_d.ap())
            xt_bufs.append(xb)

        sig_last = [None, None]
        sc_last = [None, None]
        last_sig_a = None

        pz_of = {}

        def prep_gen(blk):
            """Emit block `blk`'s input pipeline (DMA, transposes, PSUM->SBUF
            copies, xW matmuls) in small units; the caller interleaves these
            between recurrence-step emissions so the PE stream has no long
            idle stretches (keeps the HAM clock warm)."""
            t0 = blk * TS
            xt = xt_bufs[blk % 2]
            nats = []
            for r in range(2):
                nat = natp.tile([128, O], bf16, tag="nat")
                src = x_d.ap()[t0 + 4 * r : t0 + 4 * r + 4, :, :]
                cdma(out=nat[:], in_=src.rearrange("a b c -> (a b) c"))
                nats.append(nat)
            yield
            for kt in range(KT):
                cw = 80 if kt == 5 else 128
                ptr = ptrp.tile([128, 256], bf16, tag="ptr")
                for r in range(2):
                    nc.tensor.transpose(
                        ptr[0:cw, 128 * r : 128 * r + 128],
                        nats[r][:, 128 * kt : 128 * kt + cw],
                        eye[:],
                    )
                    yield
                if kt % 2 == 0:
                    nc.vector.tensor_copy(
                        xt[0:cw, kt * RB : kt * RB + RB], ptr[0:cw, :]
                    )
                else:
                    nc.scalar.copy(
                        xt[0:cw, kt * RB : kt * RB + RB], ptr[0:cw, :]
                    )
                yield
            # one PSUM bank per chain so the bank-overlap tracker never
            # serializes chain A's sigma reads against chain B's matmul
            # writes; layout per bank: gate-major, [g*128 + 16*t + b]
            pzq = []
            for q in range(GN):
                pzt = pzp.tile([128, 4 * TS * BG], f32, tag=f"pz{q}")
                pzq.append(pzt)
            pz_of[blk] = pzq
            GW = TS * BG  # 128 cols per gate region
            for q in range(GN):
                for g in range(4):
                    for kt in range(KT):
                        rview = xt[0 : KSZ[kt], kt * RB : (kt + 1) * RB]
                        rview = rview.rearrange(
                            "p (t c) -> p t c", t=TS
                        )[:, :, q * BG : (q + 1) * BG]
                        nc.tensor.matmul(
                            pzq[q][:, g * GW : (g + 1) * GW],
                            wx_sb[kt][:, g * H : (g + 1) * H],
                            rview,
                            # start=True clears has_written for the WHOLE
                            # bank: exactly once per chain-bank
                            start=(kt == 0 and g == 0),
                            stop=False,
                            skip_group_check=True,
                        )
                        yield

        for _ in prep_gen(0):
            pass

        GW = TS * BG
        for blk in range(nb):
            pzq = pz_of[blk]
            gen = prep_gen(blk + 1) if blk + 1 < nb else None

            # recurrence: 2 phase-shifted half-chains of 16; chain B emitted
            # one step behind chain A so their stages overlap across engines
            steps = []
            for t in range(TS):
                steps.append((t, 0))
                if blk == 0 and t == 0:
                    pass
                elif t == 0:
                    steps.append((TS - 1, 1, blk - 1))
                else:
                    steps.append((t - 1, 1))
            if blk == nb - 1:
                steps.append((TS - 1, 1))
            for st in steps:
                if len(st) == 3:
                    t, q, sblk = st
                    spz = pz_of[blk - 1][q]
                else:
                    t, q = st
                    spz = pzq[q]
                if True:
                    co = BG * t
                    mms = []
                    for g in range(4):
                        mm = nc.tensor.matmul(
                            spz[:, g * GW + co : g * GW + co + BG],
                            wh_b[:, g * H : (g + 1) * H],
                            hq[q][:],
                            start=False,
                            stop=(t == TS - 1),
                            skip_group_check=True,
                        )
                        mms.append(mm)

                    sig = sigp.tile([128, 4 * BG], bf16, tag=f"sig{q}")
                    zview = spz[:].rearrange("p (g c) -> p g c", g=4)[
                        :, :, co : co + BG
                    ]
                    sview = sig[:].rearrange("p (g c) -> p g c", g=4)
                    sig_i = nc.scalar.activation(sview, zview, AF.Sigmoid)
                    if q == 0:
                        last_sig_a = sig_i
                    s_i = sig[:, 0:BG]
                    s_f = sig[:, BG : 2 * BG]
                    s_o = sig[:, 2 * BG : 3 * BG]
                    s_g = sig[:, 3 * BG : 4 * BG]
                    cq = cq_t[q][:]
                    u = vecp.tile([H, BG], f32, tag=f"u{q}")
                    nc.vector._custom_dve(CMUL, out=u[:], in0=s_g, in1=s_i)
                    e = vecp.tile([H, BG], f32, tag=f"e{q}")
                    nc.vector.tensor_mul(e[:], s_f, cq)
                    nc.vector.tensor_add(cq, e[:], u[:])
                    sc = vecp.tile([H, BG], bf16, tag=f"sc{q}")
                    nc.scalar.activation(sc[:], cq, AF.Sigmoid, scale=2.0)
                    hn = hbp.tile([H, BG], bf16, tag=f"h{q}")
                    nc.vector._custom_dve(CMUL, out=hn[:], in0=sc[:], in1=s_o)
                    hq[q] = hn
                    if blk == nb - 1 and t == TS - 1:
                        sig_last[q] = sig
                        sc_last[q] = sc
                if gen is not None:
                    for _ in range(4):
                        if next(gen, "done") == "done":
                            gen = None
                            break
            while gen is not None and next(gen, "done") != "done":
                pass

        # ---- outputs ----
        hf = const.tile([H, BS], f32, tag="hf")
        hb_all = const.tile([H, BS], bf16, tag="hball")
        for q in range(GN):
            nc.vector._custom_dve(
                CMUL,
                out=hf[:, q * BG : (q + 1) * BG],
                in0=sc_last[q][:],
                in1=sig_last[q][:, 2 * BG : 3 * BG],
            )
            nc.vector.tensor_copy(hb_all[:, q * BG : (q + 1) * BG], hq[q][:])
        dma(out=oh_d.ap(), in_=hf[:])
        for q in range(GN):
            dma(out=oc_d.ap()[:, q * BG : (q + 1) * BG], in_=cq_t[q][:])

        # actor head
        p1 = phdp.tile([H, BS], f32, tag="ph")
        nc.tensor.matmul(p1[:], wa1_b[:], hb_all[:], start=True, stop=True)
        a1 = const.tile([H, BS], bf16, tag="a1")
        nc.scalar.activation(a1[:], p1[:], AF.Tanh, bias=ba1_sb[:, 0:1])
        p2 = phdp.tile([A, BS], f32, tag="ph2")
        nc.tensor.matmul(p2[:], wa2_b[:], a1[:], start=True, stop=True)
        m_sb = const.tile([A, BS], f32, tag="msb")
        nc.scalar.activation(m_sb[:], p2[:], AF.Identity, bias=ba2_sb[:, 0:1])
        dma(out=om_d.ap(), in_=m_sb[:])

        # critic head
        q1 = phdp.tile([H, BS], f32, tag="ph")
        nc.tensor.matmul(q1[:], wc1_b[:], hb_all[:], start=True, stop=True)
        c1 = const.tile([H, BS], bf16, tag="c1")
        nc.scalar.activation(c1[:], q1[:], AF.Tanh, bias=bc1_sb[:, 0:1])
        q2 = phdp.tile([1, BS], f32, tag="ph2")
        nc.tensor.matmul(q2[:], wc2_b[:], c1[:], start=True, stop=True)
        v_sb = const.tile([1, BS], f32, tag="vsb")
        nc.scalar.activation(v_sb[:], q2[:], AF.Identity, bias=bc2_sb[:, 0:1])
        dma(out=ov_d.ap(), in_=v_sb[:])

    nc.compile()
    return nc


def kernel(x, h0, c0, Wx, Wh, b, Wa1, ba1, Wa2, ba2, log_std, Wc1, bc1, Wc2, bc2):
    from concourse.bass_utils import run_bass_kernel_spmd

    x = np.asarray(x, np.float32)
    h0 = np.asarray(h0, np.float32)
    c0 = np.asarray(c0, np.float32)
    Wx = np.asarray(Wx, np.float32)
    Wh = np.asarray(Wh, np.float32)
    b = np.asarray(b, np.float32)

    # gate order in reference: [i, f, g, o]; ours: [i, f, o, g] with g scaled x2
    perm = np.concatenate(
        [np.arange(0, H), np.arange(H, 2 * H), np.arange(3 * H, 4 * H),
         np.arange(2 * H, 3 * H)]
    )
    Wxp = np.vstack(
        [Wx, np.zeros((16, 4 * H), np.float32), b[None, :],
         np.zeros((31, 4 * H), np.float32)]
    )[:, perm].copy()
    Wxp[:, 3 * H :] *= 2.0
    Whp = Wh[:, perm].copy()
    Whp[:, 3 * H :] *= 2.0

    if "nc" not in _nc_cache:
        _nc_cache["nc"] = _build()
    nc = _nc_cache["nc"]

    common = {
        "Wxp": np.ascontiguousarray(Wxp),
        "Whp": np.ascontiguousarray(Whp),
        "eye": np.eye(128, dtype=np.float32),
        "kpad": np.vstack([np.zeros((96, RB), np.float32),
                           np.ones((32, RB), np.float32)]),
        "Wa1": np.asarray(Wa1, np.float32),
        "ba1": np.asarray(ba1, np.float32).reshape(H, 1),
        "Wa2": np.asarray(Wa2, np.float32),
        "ba2": np.asarray(ba2, np.float32).reshape(A, 1),
        "Wc1": np.asarray(Wc1, np.float32),
        "bc1": np.asarray(bc1, np.float32).reshape(H, 1),
        "Wc2": np.asarray(Wc2, np.float32),
        "bc2": np.asarray(bc2, np.float32).reshape(1, 1),
        "logstd": np.asarray(log_std, np.float32).reshape(A, 1),
    }
    in_maps = []
    for c in range(NCORES):
        bs = slice(c * BS, (c + 1) * BS)
        m = dict(common)
        m["x"] = np.ascontiguousarray(x[:, bs, :])
        m["h0T"] = np.ascontiguousarray(h0[bs].T)
        m["c0T"] = np.ascontiguousarray(c0[bs].T)
        in_maps.append(m)

    res = run_bass_kernel_spmd(nc, in_maps, core_ids=list(range(NCORES)))
    rs = res.results

    hT_full = np.concatenate([rs[c]["out_h"].T for c in range(NCORES)], 0)
    cT_full = np.concatenate([rs[c]["out_c"].T for c in range(NCORES)], 0)
    mean = np.concatenate([rs[c]["out_m"].T for c in range(NCORES)], 0)
    value = np.concatenate([rs[c]["out_v"][0] for c in range(NCORES)], 0)
    std = rs[0]["out_s"][:, 0]
    return mean, std, value, hT_full, cT_full
